# revision 1
# baseline (speedup 1.0000x reference)
"""CharBiLSTM embedder on 8 TRN2 NeuronCores (Bass/Tile).

Words are sorted by length and dealt round-robin to the 8 cores; each
length-class is padded (with duplicate words) to a multiple of 8 so all
cores share an IDENTICAL sorted length profile.  Per core: 9 tiles x 512
words, grouped (2,2,2,2,1); each group's step loop runs to the group max
length.  Per tile and direction an SBUF "rhs buffer" [128, (L+1)*512] bf16
holds (from one dma_gather of a padded embedding table) the char
embeddings, a constant-1 bias row, and the running h written into slice
t+1:

  buf_f slice: h_f at partitions 0:50,  x_f at 64:114, 1.0 at 127
  buf_b slice: x_b at partitions 0:50 (1.0 at 50), h_b at 64:114

One K=128 matmul per (gate-bank, direction) accumulates W_ih@x + W_hh@h +
bias into PSUM banks [i|f|o|g], each [128, 512] with f-gates at rows 0:50
and b-gates at rows 64:114.  ScalarE: one sigmoid over banks i,f,o + tanh
on g + tanh on c; VectorE: 4 bf16 tensor ops per step.  Because the length
profile is identical on every core, the final h of the words with length l
occupy a static contiguous column range of slice l — extraction is a few
static copies per tile.
"""
import os
import sys

os.environ.setdefault("CONCOURSE_SCRUB_NEFF_DEBUG_INFO", "1")
sys.path.insert(0, "/opt/trn_rl_repo")

from contextlib import ExitStack

import ml_dtypes
import numpy as np

import concourse.bass as bass
import concourse.mybir as mybir
import concourse.tile as tile
from concourse import bacc
from concourse.tile import add_dep_helper
from concourse.bass_utils import run_bass_kernel_spmd

N, T, E, H, V = 32768, 20, 50, 50, 200
NCORES = 8
NT = 512                  # words per tile
NTILES = 9
NWPAD = NT * NTILES       # padded words per core
GROUPS = ((0, 1), (2, 3), (4, 5), (6, 7), (8,))
BF16 = mybir.dt.bfloat16
F32 = mybir.dt.float32
I16 = mybir.dt.int16
MAXL = T + 1              # slices per tile <= T+1

AF = mybir.ActivationFunctionType
SIG = AF.Sigmoid
TANH = AF.Tanh


def build_graph(Ls, Ltl, ranges):
    """Ls: per-group max length; Ltl: per-tile max; ranges: (l,a,b) runs."""
    nc = bacc.Bacc()
    wts_ext = nc.declare_dram_parameter("wts", [8, 128, 128], BF16, isOutput=False)
    tab_ext = nc.declare_dram_parameter("tab", [2, 128, 256], BF16, isOutput=False)
    gidx_ext = nc.declare_dram_parameter(
        "gidx", [NTILES, 2, 128, MAXL * (NT // 16)], I16, isOutput=False
    )
    out_ext = nc.declare_dram_parameter("out", [100, NWPAD], F32, isOutput=True)

    with tile.TileContext(nc) as tc, ExitStack() as ctx:
        cpool = ctx.enter_context(tc.tile_pool(name="const", bufs=1))
        bpool_f = ctx.enter_context(tc.tile_pool(name="buf_f", bufs=3))
        bpool_b = ctx.enter_context(tc.tile_pool(name="buf_b", bufs=3))
        ipool = ctx.enter_context(tc.tile_pool(name="idx", bufs=4))
        pspool = ctx.enter_context(tc.tile_pool(name="ps", bufs=1, space="PSUM"))

        wts_sb = cpool.tile([128, 8 * 128], BF16, tag="wts", name="wts_sb")
        nc.gpsimd.dma_start(
            wts_sb[:].rearrange("k (i m) -> k i m", i=8),
            wts_ext[:].rearrange("i k m -> k i m"),
        )
        tab_sb = cpool.tile([128, 512], BF16, tag="tab", name="tab_sb")
        nc.gpsimd.dma_start(
            tab_sb[:].rearrange("k (i m) -> k i m", i=2),
            tab_ext[:].rearrange("i k m -> k i m"),
        )
        tabs = (tab_sb[:, 0:256], tab_sb[:, 256:512])
        hall = cpool.tile([128, NTILES * NT], BF16, tag="hall", name="hall")
        nc.vector.memset(hall[:], 0.0)
        NA = NTILES  # position slots
        tcp_all = cpool.tile([128, NA * 2 * NT], BF16, tag="tcpa", name="tcp_all")
        nc.vector.memset(tcp_all[:], 0.0)
        sig_all = cpool.tile([128, NA * 3 * NT], BF16, tag="siga", name="sig_all")
        thc_all = cpool.tile([128, NA * NT], BF16, tag="thca", name="thc_all")
        tmp_all = cpool.tile([128, 4 * NT], BF16, tag="tmpa", name="tmp_all")
        tcp3 = tcp_all[:].rearrange("p (s w) -> p s w", w=2 * NT)
        sig3 = sig_all[:].rearrange("p (s w) -> p s w", w=3 * NT)
        thc3 = thc_all[:].rearrange("p (s w) -> p s w", w=NT)
        tmp3 = tmp_all[:].rearrange("p (s w) -> p s w", w=2 * NT)
        ps_all = pspool.tile([128, 8 * NT], F32, tag="psa", name="ps_all")

        tactive = [tl for tl in range(NTILES) if Ltl[tl] > 0]
        starts = {}
        for j, tl in enumerate(tactive):
            if j < 2:
                starts[tl] = 0
            else:
                prev2 = tactive[j - 2]
                starts[tl] = starts[prev2] + Ltl[prev2]
        S = max(starts[tl] + Ltl[tl] for tl in tactive)

        bufs = {}
        for tl in tactive:
            Lt = Ltl[tl]
            W16 = Lt * (NT // 16)
            idx_sb = ipool.tile(
                [128, 2 * MAXL * (NT // 16)], I16, tag="idx", name="idxt"
            )
            nc.gpsimd.dma_start(
                idx_sb[:, : 2 * W16].rearrange("p (d w) -> p d w", d=2),
                gidx_ext[tl, :, :, :W16].rearrange("d p w -> p d w"),
            )
            for d, pool in ((0, bpool_f), (1, bpool_b)):
                buf = pool.tile([128, MAXL * NT], BF16, tag=f"buf{d}", name=f"buf{d}")
                nc.gpsimd.dma_gather(
                    out_ap=buf[:, : Lt * NT].rearrange("p (o n) -> p o n", o=1),
                    in_ap=tabs[d],
                    idxs_ap=idx_sb[:, d * W16 : (d + 1) * W16],
                    num_idxs=Lt * NT,
                    num_idxs_reg=Lt * NT,
                    elem_size=128,
                    transpose=True,
                    sbuf_tokens_per_rank=128,
                    sbuf_free_dim_per_rank=256,
                    sbuf_free_dim_pad_per_rank=0,
                    sbuf_byte_offset=0,
                    single_packet=False,
                )
                bufs[(tl, d)] = buf


        for s in range(S):
            act = [tl for tl in tactive if starts[tl] <= s < starts[tl] + Ltl[tl]]
            loc = {tl: s - starts[tl] for tl in act}
            slot = {tl: j for j, tl in enumerate(act)}
            prev_mm = None
            worder = [(b, h) for b in range(4) for h in range(2)]
            fh = 0 if s % 2 == 0 else 1     # first-emitted half per bank
            if s % 2 == 1:
                worder = worder[::-1]
            for b, half in worder:
                w_ap = wts_sb[:, (2 * b + half) * 128 : (2 * b + half + 1) * 128]
                for tl in act:
                    t = loc[tl]
                    j = slot[tl]
                    off = (3 * j + b) * NT if b < 3 else (6 + j) * NT
                    rhs = bufs[(tl, half)][:, t * NT : (t + 1) * NT]
                    mm = nc.tensor.matmul(
                        ps_all[:, off : off + NT],
                        w_ap,
                        rhs,
                        start=(half == fh),
                        stop=(half == 1 - fh),
                    )
                    if prev_mm is not None:
                        add_dep_helper(
                            mm.ins, prev_mm.ins, sync=False,
                            reason="weight-major PE order",
                        )
                    prev_mm = mm
            pos = {tl: tactive.index(tl) for tl in act}
            paired = len(act) == 2 and pos[act[1]] == pos[act[0]] + 1
            k0 = pos[act[0]]
            if paired:
                nc.scalar.activation(
                    sig_all[:, k0 * 3 * NT : (k0 + 2) * 3 * NT],
                    ps_all[:, 0 : 6 * NT], SIG,
                )
                nc.scalar.activation(
                    tcp3[:, k0 : k0 + 2, 0:NT],
                    ps_all[:].rearrange("p (s w) -> p s w", w=NT)[:, 6:8, :],
                    TANH,
                )
            else:
                for tl in act:
                    k = pos[tl]
                    j = slot[tl]
                    nc.scalar.activation(
                        sig_all[:, k * 3 * NT : (k + 1) * 3 * NT],
                        ps_all[:, 3 * j * NT : (3 * j + 3) * NT], SIG,
                    )
                    nc.scalar.activation(
                        tcp_all[:, k * 2 * NT : k * 2 * NT + NT],
                        ps_all[:, (6 + j) * NT : (7 + j) * NT], TANH,
                    )
            if paired:
                nc.vector.tensor_mul(
                    tmp3[:, 0:2, :],
                    tcp3[:, k0 : k0 + 2, :],
                    sig3[:, k0 : k0 + 2, 0 : 2 * NT],
                )
                nc.vector.tensor_add(
                    tcp3[:, k0 : k0 + 2, NT : 2 * NT],
                    tmp3[:, 0:2, 0:NT],
                    tmp3[:, 0:2, NT : 2 * NT],
                )
                nc.scalar.activation(
                    thc3[:, k0 : k0 + 2, :],
                    tcp3[:, k0 : k0 + 2, NT : 2 * NT], TANH,
                )
            else:
                for tl in act:
                    k = pos[tl]
                    nc.vector.tensor_mul(
                        tmp_all[:, 0 : 2 * NT],
                        tcp_all[:, k * 2 * NT : (k + 1) * 2 * NT],
                        sig_all[:, k * 3 * NT : k * 3 * NT + 2 * NT],
                    )
                    nc.vector.tensor_add(
                        tcp_all[:, k * 2 * NT + NT : (k + 1) * 2 * NT],
                        tmp_all[:, 0:NT],
                        tmp_all[:, NT : 2 * NT],
                    )
                    nc.scalar.activation(
                        thc_all[:, k * NT : (k + 1) * NT],
                        tcp_all[:, k * 2 * NT + NT : (k + 1) * 2 * NT], TANH,
                    )
            for tl in act:
                t = loc[tl]
                k = pos[tl]
                nc.vector.tensor_mul(
                    bufs[(tl, 0)][0:50, (t + 1) * NT : (t + 2) * NT],
                    sig_all[0:50, k * 3 * NT + 2 * NT : (k + 1) * 3 * NT],
                    thc_all[0:50, k * NT : (k + 1) * NT],
                )
                nc.vector.tensor_mul(
                    bufs[(tl, 1)][64:114, (t + 1) * NT : (t + 2) * NT],
                    sig_all[64:114, k * 3 * NT + 2 * NT : (k + 1) * 3 * NT],
                    thc_all[64:114, k * NT : (k + 1) * NT],
                )
            for tl in act:
                if loc[tl] == Ltl[tl] - 1:
                    for (l, a, b2) in ranges[tl]:
                        nc.vector.tensor_copy(
                            hall[0:50, tl * NT + a : tl * NT + b2],
                            bufs[(tl, 0)][0:50, l * NT + a : l * NT + b2],
                        )
                        nc.vector.tensor_copy(
                            hall[64:114, tl * NT + a : tl * NT + b2],
                            bufs[(tl, 1)][64:114, l * NT + a : l * NT + b2],
                        )
        nc.gpsimd.dma_start(out_ext[0:50, :], hall[0:50, :])
        nc.gpsimd.dma_start(out_ext[50:100, :], hall[64:114, :])
    nc.finalize()
    _dedup_ldweights(nc)
    return nc


def _dedup_ldweights(nc):
    """Drop consecutive PE Ldweights that reload the identical stationary."""
    PE = mybir.EngineType.PE
    removed = 0
    for blk in nc.m.functions[0].blocks:
        il = blk.instructions
        cur = None
        drop = []
        for idx, inst in enumerate(il):
            if getattr(inst, "engine", None) != PE:
                continue
            if type(inst).__name__ != "InstLdweights":
                continue
            key = repr(inst.ins[0])
            si = inst.sync_info
            waits = list(si.on_wait) if si is not None else []
            upds = list(si.on_update) if si is not None else []
            if key == cur and not upds:
                if not waits:
                    drop.append(idx)
                    continue
                nxt = None
                for j in range(idx + 1, len(il)):
                    if getattr(il[j], "engine", None) == PE:
                        nxt = il[j]
                        break
                if nxt is not None:
                    nsi = nxt.sync_info
                    nwaits = list(nsi.on_wait) if nsi is not None else []
                    if len(nwaits) + len(waits) <= 1:
                        if nsi is None:
                            nxt.sync_info = mybir.SyncInfo(
                                on_wait=waits, on_update=[]
                            )
                        else:
                            nsi.on_wait = nwaits + waits
                        drop.append(idx)
                        continue
                continue
            cur = key
        for idx in reversed(drop):
            del il[idx]
        removed += len(drop)
    return removed


def prepare_host(inputs):
    ci = np.asarray(inputs["char_indices"])
    lens = np.asarray(inputs["word_lengths"]).astype(np.int64)
    emb = np.array(inputs["emb"], dtype=np.float32)
    emb[0] = 0.0

    # --- padded, sorted word list with per-core-identical length profile ---
    order = np.argsort(lens, kind="stable")
    counts = np.bincount(lens, minlength=T + 1)
    dup_ids = []
    for l in range(T + 1):
        rem = counts[l] % 8
        if rem:
            w = order[np.searchsorted(lens[order], l)]
            dup_ids += [w] * (8 - rem)
    front = NWPAD * NCORES - N - len(dup_ids)
    assert front >= 0 and front % 8 == 0
    shortest = order[0]
    all_ids = np.concatenate(
        [order, np.array(dup_ids + [shortest] * front, dtype=np.int64)]
    )
    words_pad = all_ids[np.argsort(lens[all_ids], kind="stable")]
    plens = lens[words_pad]
    assert (plens.reshape(-1, 8).max(1) == plens.reshape(-1, 8).min(1)).all()
    prof = plens[::8].astype(np.int64)          # per-core length profile [NWPAD]

    Ls = tuple(int(prof[(tiles[-1] + 1) * NT - 1]) for tiles in GROUPS)
    Ltl = tuple(int(prof[(tl + 1) * NT - 1]) for tl in range(NTILES))
    ranges = []
    for tl in range(NTILES):
        seg = prof[tl * NT : (tl + 1) * NT]
        runs = []
        a = 0
        for p in range(1, NT + 1):
            if p == NT or seg[p] != seg[a]:
                runs.append((int(seg[a]), a, p))
                a = p
        ranges.append(tuple(runs))
    ranges = tuple(ranges)

    # --- weights: bank order i, f, o, g -> torch gate-row order i, f, g, o ---
    rows = {0: slice(0, 50), 1: slice(50, 100), 2: slice(150, 200), 3: slice(100, 150)}
    wts = np.zeros((8, 128, 128), np.float32)
    for b in range(4):
        r = rows[b]
        for half, sfx in enumerate("fb"):
            Wih = np.asarray(inputs[f"W_ih_{sfx}"], dtype=np.float32)
            Whh = np.asarray(inputs[f"W_hh_{sfx}"], dtype=np.float32)
            bias = np.asarray(inputs[f"b_ih_{sfx}"], dtype=np.float32) + np.asarray(
                inputs[f"b_hh_{sfx}"], dtype=np.float32
            )
            w = wts[2 * b + half]
            if half == 0:   # f-dir: h at K 0:50, x at K 64:114, 1.0 at K 127
                w[0:50, 0:50] = Whh[r].T
                w[64:114, 0:50] = Wih[r].T
                w[127, 0:50] = bias[r]
            else:           # b-dir: x at K 0:50, 1.0 at K 50, h at K 64:114
                w[0:50, 64:114] = Wih[r].T
                w[50, 64:114] = bias[r]
                w[64:114, 64:114] = Whh[r].T
    wts_bf = wts.astype(ml_dtypes.bfloat16)

    tab = np.zeros((2, 128, 256), np.float32)
    for v in range(V):
        rank, tok = v // 128, v % 128
        tab[0, tok, rank * 128 + 64 : rank * 128 + 114] = emb[v]
        tab[0, tok, rank * 128 + 127] = 1.0
        tab[1, tok, rank * 128 + 0 : rank * 128 + 50] = emb[v]
        tab[1, tok, rank * 128 + 50] = 1.0
    tab_bf = tab.astype(ml_dtypes.bfloat16)

    def wrap128(flat):
        # [L*NT] -> [128, L*NT//16]: wrapped in 16 partitions, replicated x8
        a = flat.reshape(-1, 16).T.astype(np.int16)
        return np.tile(a, (8, 1))

    in_maps = []
    cores_meta = []
    for c in range(NCORES):
        widx = words_pad[c::NCORES]
        ci_c = ci[widx]
        len_c = lens[widx]
        gidx = np.zeros((NTILES, 2, 128, MAXL * (NT // 16)), np.int16)
        for tl in range(NTILES):
            Lg = Ltl[tl]
            if Lg == 0:
                continue
            cw = ci_c[tl * NT : (tl + 1) * NT]          # [NT, T]
            lw = len_c[tl * NT : (tl + 1) * NT]          # [NT]
            tt = np.arange(Lg)
            f_chars = cw[:, :Lg].T                       # [Lg, NT]
            b_pos = np.maximum(lw[None, :] - 1 - tt[:, None], 0)
            b_chars = cw[np.arange(NT)[None, :], b_pos]  # [Lg, NT]
            gidx[tl, 0, :, : Lg * (NT // 16)] = wrap128(f_chars.reshape(-1))
            gidx[tl, 1, :, : Lg * (NT // 16)] = wrap128(b_chars.reshape(-1))
        in_maps.append({"wts": wts_bf, "tab": tab_bf, "gidx": gidx})
        cores_meta.append(widx)
    return Ls, Ltl, ranges, in_maps, cores_meta


_GRAPH_CACHE = {}
TRACE = False
LAST_RESULT = None


def kernel(**inputs):
    Ls, Ltl, ranges, in_maps, cores_meta = prepare_host(inputs)
    key = (Ls, Ltl, ranges)
    if key not in _GRAPH_CACHE:
        _GRAPH_CACHE[key] = build_graph(Ls, Ltl, ranges)
    nc = _GRAPH_CACHE[key]
    global LAST_RESULT
    res = run_bass_kernel_spmd(
        nc, in_maps, core_ids=list(range(NCORES)), trace=TRACE
    )
    LAST_RESULT = res
    out = np.zeros((N, 2 * H), np.float32)
    for c in range(NCORES):
        out[cores_meta[c]] = res.results[c]["out"].T
    return out



# revision 6
# speedup vs baseline: 2.1913x; 2.1913x over previous
"""CharBiLSTM embedder on 8 TRN2 NeuronCores (Bass/Tile).

Words are sorted by length and dealt round-robin to the 8 cores; each
length-class is padded (with duplicate words) to a multiple of 8 so all
cores share an IDENTICAL sorted length profile.  Per core: 9 tiles x 512
words, grouped (2,2,2,2,1); each group's step loop runs to the group max
length.  Per tile and direction an SBUF "rhs buffer" [128, (L+1)*512] bf16
holds (from one dma_gather of a padded embedding table) the char
embeddings, a constant-1 bias row, and the running h written into slice
t+1:

  buf_f slice: h_f at partitions 0:50,  x_f at 64:114, 1.0 at 127
  buf_b slice: x_b at partitions 0:50 (1.0 at 50), h_b at 64:114

One K=128 matmul per (gate-bank, direction) accumulates W_ih@x + W_hh@h +
bias into PSUM banks [i|f|o|g], each [128, 512] with f-gates at rows 0:50
and b-gates at rows 64:114.  ScalarE: one sigmoid over banks i,f,o + tanh
on g + tanh on c; VectorE: 4 bf16 tensor ops per step.  Because the length
profile is identical on every core, the final h of the words with length l
occupy a static contiguous column range of slice l — extraction is a few
static copies per tile.
"""
import os
import sys

os.environ.setdefault("CONCOURSE_SCRUB_NEFF_DEBUG_INFO", "1")
sys.path.insert(0, "/opt/trn_rl_repo")

from contextlib import ExitStack

import ml_dtypes
import numpy as np

import concourse.bass as bass
import concourse.mybir as mybir
import concourse.tile as tile
from concourse import bacc
from concourse.tile import add_dep_helper
from concourse.bass_utils import run_bass_kernel_spmd

N, T, E, H, V = 32768, 20, 50, 50, 200
NCORES = 8
NT = 512                  # words per tile
NTILES = 9
NWPAD = NT * NTILES       # padded words per core
GROUPS = ((0, 1), (2, 3), (4, 5), (6, 7), (8,))
BF16 = mybir.dt.bfloat16
F32 = mybir.dt.float32
I16 = mybir.dt.int16
MAXL = T + 1              # slices per tile <= T+1

AF = mybir.ActivationFunctionType
SIG = AF.Sigmoid
TANH = AF.Tanh


def build_graph(Ls, Ltl, ranges):
    """Ls: per-group max length; Ltl: per-tile max; ranges: (l,a,b) runs."""
    nc = bacc.Bacc()
    wts_ext = nc.declare_dram_parameter("wts", [8, 128, 128], BF16, isOutput=False)
    tab_ext = nc.declare_dram_parameter("tab", [2, 128, 256], BF16, isOutput=False)
    gidx_ext = nc.declare_dram_parameter(
        "gidx", [NTILES, 2, 16, MAXL * (NT // 16)], I16, isOutput=False
    )
    out_ext = nc.declare_dram_parameter("out", [100, NWPAD], BF16, isOutput=True)

    with tile.TileContext(nc) as tc, ExitStack() as ctx:
        cpool = ctx.enter_context(tc.tile_pool(name="const", bufs=1))
        bpool_f = ctx.enter_context(tc.tile_pool(name="buf_f", bufs=3))
        bpool_b = ctx.enter_context(tc.tile_pool(name="buf_b", bufs=3))
        ipool = ctx.enter_context(tc.tile_pool(name="idx", bufs=4))
        pspool = ctx.enter_context(tc.tile_pool(name="ps", bufs=1, space="PSUM"))

        wts_sb = cpool.tile([128, 8 * 128], BF16, tag="wts", name="wts_sb")
        nc.gpsimd.dma_start(
            wts_sb[:].rearrange("k (i m) -> k i m", i=8),
            wts_ext[:].rearrange("i k m -> k i m"),
        )
        tab_sb = cpool.tile([128, 512], BF16, tag="tab", name="tab_sb")
        nc.gpsimd.dma_start(
            tab_sb[:].rearrange("k (i m) -> k i m", i=2),
            tab_ext[:].rearrange("i k m -> k i m"),
        )
        tabs = (tab_sb[:, 0:256], tab_sb[:, 256:512])
        hall = cpool.tile([128, NTILES * NT], BF16, tag="hall", name="hall")
        nc.vector.memset(hall[:], 0.0)
        NA = NTILES  # position slots
        tcp_all = cpool.tile([128, NA * 2 * NT], BF16, tag="tcpa", name="tcp_all")
        nc.vector.memset(tcp_all[:], 0.0)
        sig_all = cpool.tile([128, NA * 3 * NT], BF16, tag="siga", name="sig_all")
        thc_all = cpool.tile([128, NA * NT], BF16, tag="thca", name="thc_all")
        tmp_all = cpool.tile([128, 4 * NT], BF16, tag="tmpa", name="tmp_all")
        tcp3 = tcp_all[:].rearrange("p (s w) -> p s w", w=2 * NT)
        sig3 = sig_all[:].rearrange("p (s w) -> p s w", w=3 * NT)
        thc3 = thc_all[:].rearrange("p (s w) -> p s w", w=NT)
        tmp3 = tmp_all[:].rearrange("p (s w) -> p s w", w=2 * NT)
        ps_all = pspool.tile([128, 8 * NT], F32, tag="psa", name="ps_all")

        tactive = [tl for tl in range(NTILES) if Ltl[tl] > 0]
        starts = {}
        for j, tl in enumerate(tactive):
            if j < 2:
                starts[tl] = 0
            else:
                prev2 = tactive[j - 2]
                starts[tl] = starts[prev2] + Ltl[prev2]
        S = max(starts[tl] + Ltl[tl] for tl in tactive)

        bufs = {}
        for tl in tactive:
            Lt = Ltl[tl]
            W16 = Lt * (NT // 16)
            idx_sb = ipool.tile(
                [128, 2 * MAXL * (NT // 16)], I16, tag="idx", name="idxt"
            )
            nc.gpsimd.dma_start(
                idx_sb[0:16, : 2 * W16].rearrange("p (d w) -> p d w", d=2),
                gidx_ext[tl, :, :, :W16].rearrange("d p w -> p d w"),
            )
            # replicate the 16-partition wrap to all 128 partitions on-device
            # (DGE desc-gen cores each read their own 16-partition replica)
            nc.gpsimd.dma_start(idx_sb[16:32, : 2 * W16], idx_sb[0:16, : 2 * W16])
            nc.gpsimd.dma_start(idx_sb[32:64, : 2 * W16], idx_sb[0:32, : 2 * W16])
            nc.gpsimd.dma_start(idx_sb[64:128, : 2 * W16], idx_sb[0:64, : 2 * W16])
            for d, pool in ((0, bpool_f), (1, bpool_b)):
                buf = pool.tile([128, MAXL * NT], BF16, tag=f"buf{d}", name=f"buf{d}")
                nc.gpsimd.dma_gather(
                    out_ap=buf[:, : Lt * NT].rearrange("p (o n) -> p o n", o=1),
                    in_ap=tabs[d],
                    idxs_ap=idx_sb[:, d * W16 : (d + 1) * W16],
                    num_idxs=Lt * NT,
                    num_idxs_reg=Lt * NT,
                    elem_size=128,
                    transpose=True,
                    sbuf_tokens_per_rank=128,
                    sbuf_free_dim_per_rank=256,
                    sbuf_free_dim_pad_per_rank=0,
                    sbuf_byte_offset=0,
                    single_packet=False,
                )
                bufs[(tl, d)] = buf


        for s in range(S):
            act = [tl for tl in tactive if starts[tl] <= s < starts[tl] + Ltl[tl]]
            loc = {tl: s - starts[tl] for tl in act}
            slot = {tl: j for j, tl in enumerate(act)}
            prev_mm = None
            worder = [(b, h) for b in range(4) for h in range(2)]
            fh = 0 if s % 2 == 0 else 1     # first-emitted half per bank
            if s % 2 == 1:
                worder = worder[::-1]
            for b, half in worder:
                w_ap = wts_sb[:, (2 * b + half) * 128 : (2 * b + half + 1) * 128]
                for tl in act:
                    t = loc[tl]
                    j = slot[tl]
                    off = (3 * j + b) * NT if b < 3 else (6 + j) * NT
                    rhs = bufs[(tl, half)][:, t * NT : (t + 1) * NT]
                    mm = nc.tensor.matmul(
                        ps_all[:, off : off + NT],
                        w_ap,
                        rhs,
                        start=(half == fh),
                        stop=(half == 1 - fh),
                    )
                    if prev_mm is not None:
                        add_dep_helper(
                            mm.ins, prev_mm.ins, sync=False,
                            reason="weight-major PE order",
                        )
                    prev_mm = mm
            pos = {tl: tactive.index(tl) for tl in act}
            paired = len(act) == 2 and pos[act[1]] == pos[act[0]] + 1
            k0 = pos[act[0]]
            if paired:
                nc.scalar.activation(
                    sig_all[:, k0 * 3 * NT : (k0 + 2) * 3 * NT],
                    ps_all[:, 0 : 6 * NT], SIG,
                )
                nc.scalar.activation(
                    tcp3[:, k0 : k0 + 2, 0:NT],
                    ps_all[:].rearrange("p (s w) -> p s w", w=NT)[:, 6:8, :],
                    TANH,
                )
            else:
                for tl in act:
                    k = pos[tl]
                    j = slot[tl]
                    nc.scalar.activation(
                        sig_all[:, k * 3 * NT : (k + 1) * 3 * NT],
                        ps_all[:, 3 * j * NT : (3 * j + 3) * NT], SIG,
                    )
                    nc.scalar.activation(
                        tcp_all[:, k * 2 * NT : k * 2 * NT + NT],
                        ps_all[:, (6 + j) * NT : (7 + j) * NT], TANH,
                    )
            if paired:
                nc.vector.tensor_mul(
                    tmp3[:, 0:2, :],
                    tcp3[:, k0 : k0 + 2, :],
                    sig3[:, k0 : k0 + 2, 0 : 2 * NT],
                )
                nc.vector.tensor_add(
                    tcp3[:, k0 : k0 + 2, NT : 2 * NT],
                    tmp3[:, 0:2, 0:NT],
                    tmp3[:, 0:2, NT : 2 * NT],
                )
                nc.scalar.activation(
                    thc3[:, k0 : k0 + 2, :],
                    tcp3[:, k0 : k0 + 2, NT : 2 * NT], TANH,
                )
            else:
                for tl in act:
                    k = pos[tl]
                    nc.vector.tensor_mul(
                        tmp_all[:, 0 : 2 * NT],
                        tcp_all[:, k * 2 * NT : (k + 1) * 2 * NT],
                        sig_all[:, k * 3 * NT : k * 3 * NT + 2 * NT],
                    )
                    nc.vector.tensor_add(
                        tcp_all[:, k * 2 * NT + NT : (k + 1) * 2 * NT],
                        tmp_all[:, 0:NT],
                        tmp_all[:, NT : 2 * NT],
                    )
                    nc.scalar.activation(
                        thc_all[:, k * NT : (k + 1) * NT],
                        tcp_all[:, k * 2 * NT + NT : (k + 1) * 2 * NT], TANH,
                    )
            for tl in act:
                t = loc[tl]
                k = pos[tl]
                nc.vector.tensor_mul(
                    bufs[(tl, 0)][0:50, (t + 1) * NT : (t + 2) * NT],
                    sig_all[0:50, k * 3 * NT + 2 * NT : (k + 1) * 3 * NT],
                    thc_all[0:50, k * NT : (k + 1) * NT],
                )
                nc.vector.tensor_mul(
                    bufs[(tl, 1)][64:114, (t + 1) * NT : (t + 2) * NT],
                    sig_all[64:114, k * 3 * NT + 2 * NT : (k + 1) * 3 * NT],
                    thc_all[64:114, k * NT : (k + 1) * NT],
                )
            for tl in act:
                if loc[tl] == Ltl[tl] - 1:
                    for (l, a, b2) in ranges[tl]:
                        nc.vector.tensor_copy(
                            hall[0:50, tl * NT + a : tl * NT + b2],
                            bufs[(tl, 0)][0:50, l * NT + a : l * NT + b2],
                        )
                        nc.vector.tensor_copy(
                            hall[64:114, tl * NT + a : tl * NT + b2],
                            bufs[(tl, 1)][64:114, l * NT + a : l * NT + b2],
                        )
        nc.gpsimd.dma_start(out_ext[0:50, :], hall[0:50, :])
        nc.gpsimd.dma_start(out_ext[50:100, :], hall[64:114, :])
    nc.finalize()
    _dedup_ldweights(nc)
    return nc


def _dedup_ldweights(nc):
    """Drop consecutive PE Ldweights that reload the identical stationary."""
    PE = mybir.EngineType.PE
    removed = 0
    for blk in nc.m.functions[0].blocks:
        il = blk.instructions
        cur = None
        drop = []
        for idx, inst in enumerate(il):
            if getattr(inst, "engine", None) != PE:
                continue
            if type(inst).__name__ != "InstLdweights":
                continue
            key = repr(inst.ins[0])
            si = inst.sync_info
            waits = list(si.on_wait) if si is not None else []
            upds = list(si.on_update) if si is not None else []
            if key == cur and not upds:
                if not waits:
                    drop.append(idx)
                    continue
                nxt = None
                for j in range(idx + 1, len(il)):
                    if getattr(il[j], "engine", None) == PE:
                        nxt = il[j]
                        break
                if nxt is not None:
                    nsi = nxt.sync_info
                    nwaits = list(nsi.on_wait) if nsi is not None else []
                    if len(nwaits) + len(waits) <= 1:
                        if nsi is None:
                            nxt.sync_info = mybir.SyncInfo(
                                on_wait=waits, on_update=[]
                            )
                        else:
                            nsi.on_wait = nwaits + waits
                        drop.append(idx)
                        continue
                continue
            cur = key
        for idx in reversed(drop):
            del il[idx]
        removed += len(drop)
    return removed


def prepare_host(inputs):
    ci = np.asarray(inputs["char_indices"])
    lens = np.asarray(inputs["word_lengths"]).astype(np.int64)
    emb = np.array(inputs["emb"], dtype=np.float32)
    emb[0] = 0.0

    # --- padded, sorted word list with per-core-identical length profile ---
    order = np.argsort(lens, kind="stable")
    counts = np.bincount(lens, minlength=T + 1)
    dup_ids = []
    for l in range(T + 1):
        rem = counts[l] % 8
        if rem:
            w = order[np.searchsorted(lens[order], l)]
            dup_ids += [w] * (8 - rem)
    front = NWPAD * NCORES - N - len(dup_ids)
    assert front >= 0 and front % 8 == 0
    shortest = order[0]
    all_ids = np.concatenate(
        [order, np.array(dup_ids + [shortest] * front, dtype=np.int64)]
    )
    words_pad = all_ids[np.argsort(lens[all_ids], kind="stable")]
    plens = lens[words_pad]
    assert (plens.reshape(-1, 8).max(1) == plens.reshape(-1, 8).min(1)).all()
    prof = plens[::8].astype(np.int64)          # per-core length profile [NWPAD]

    Ls = tuple(int(prof[(tiles[-1] + 1) * NT - 1]) for tiles in GROUPS)
    Ltl = tuple(int(prof[(tl + 1) * NT - 1]) for tl in range(NTILES))
    ranges = []
    for tl in range(NTILES):
        seg = prof[tl * NT : (tl + 1) * NT]
        runs = []
        a = 0
        for p in range(1, NT + 1):
            if p == NT or seg[p] != seg[a]:
                runs.append((int(seg[a]), a, p))
                a = p
        ranges.append(tuple(runs))
    ranges = tuple(ranges)

    # --- weights: bank order i, f, o, g -> torch gate-row order i, f, g, o ---
    rows = {0: slice(0, 50), 1: slice(50, 100), 2: slice(150, 200), 3: slice(100, 150)}
    wts = np.zeros((8, 128, 128), np.float32)
    for b in range(4):
        r = rows[b]
        for half, sfx in enumerate("fb"):
            Wih = np.asarray(inputs[f"W_ih_{sfx}"], dtype=np.float32)
            Whh = np.asarray(inputs[f"W_hh_{sfx}"], dtype=np.float32)
            bias = np.asarray(inputs[f"b_ih_{sfx}"], dtype=np.float32) + np.asarray(
                inputs[f"b_hh_{sfx}"], dtype=np.float32
            )
            w = wts[2 * b + half]
            if half == 0:   # f-dir: h at K 0:50, x at K 64:114, 1.0 at K 127
                w[0:50, 0:50] = Whh[r].T
                w[64:114, 0:50] = Wih[r].T
                w[127, 0:50] = bias[r]
            else:           # b-dir: x at K 0:50, 1.0 at K 50, h at K 64:114
                w[0:50, 64:114] = Wih[r].T
                w[50, 64:114] = bias[r]
                w[64:114, 64:114] = Whh[r].T
    wts_bf = wts.astype(ml_dtypes.bfloat16)

    tab = np.zeros((2, 128, 256), np.float32)
    for v in range(V):
        rank, tok = v // 128, v % 128
        tab[0, tok, rank * 128 + 64 : rank * 128 + 114] = emb[v]
        tab[0, tok, rank * 128 + 127] = 1.0
        tab[1, tok, rank * 128 + 0 : rank * 128 + 50] = emb[v]
        tab[1, tok, rank * 128 + 50] = 1.0
    tab_bf = tab.astype(ml_dtypes.bfloat16)

    def wrap128(flat):
        # [L*NT] -> [16, L*NT//16]: wrapped in 16 partitions (replication to
        # 128 partitions happens on-device via doubling DMAs)
        return flat.reshape(-1, 16).T.astype(np.int16)

    in_maps = []
    cores_meta = []
    for c in range(NCORES):
        widx = words_pad[c::NCORES]
        ci_c = ci[widx]
        len_c = lens[widx]
        gidx = np.zeros((NTILES, 2, 16, MAXL * (NT // 16)), np.int16)
        for tl in range(NTILES):
            Lg = Ltl[tl]
            if Lg == 0:
                continue
            cw = ci_c[tl * NT : (tl + 1) * NT]          # [NT, T]
            lw = len_c[tl * NT : (tl + 1) * NT]          # [NT]
            tt = np.arange(Lg)
            f_chars = cw[:, :Lg].T                       # [Lg, NT]
            b_pos = np.maximum(lw[None, :] - 1 - tt[:, None], 0)
            b_chars = cw[np.arange(NT)[None, :], b_pos]  # [Lg, NT]
            gidx[tl, 0, :, : Lg * (NT // 16)] = wrap128(f_chars.reshape(-1))
            gidx[tl, 1, :, : Lg * (NT // 16)] = wrap128(b_chars.reshape(-1))
        in_maps.append({"wts": wts_bf, "tab": tab_bf, "gidx": gidx})
        cores_meta.append(widx)
    return Ls, Ltl, ranges, in_maps, cores_meta


_GRAPH_CACHE = {}
TRACE = False
LAST_RESULT = None


def kernel(**inputs):
    Ls, Ltl, ranges, in_maps, cores_meta = prepare_host(inputs)
    key = (Ls, Ltl, ranges)
    if key not in _GRAPH_CACHE:
        _GRAPH_CACHE[key] = build_graph(Ls, Ltl, ranges)
    nc = _GRAPH_CACHE[key]
    global LAST_RESULT
    res = run_bass_kernel_spmd(
        nc, in_maps, core_ids=list(range(NCORES)), trace=TRACE
    )
    LAST_RESULT = res
    out = np.zeros((N, 2 * H), np.float32)
    for c in range(NCORES):
        out[cores_meta[c]] = res.results[c]["out"].T.astype(np.float32)
    return out

